# revision 27
# baseline (speedup 1.0000x reference)
"""Trainium2 Bass kernel for nn_BiAttention (dense_transformer).

Reference computation (per batch b, with J = question_ends+1 = 64,
T = L - J = 1984, H = 1024):
    query   = enc[:J]                      (J, H)
    context = enc[J:]                      (T, H)
    w_c, w_q, w_m = w[:H], w[H:2H], w[2H:]
    S[t,j]  = ctx[t]@w_c + q[j]@w_q + sum_h ctx[t,h]*w_m[h]*q[j,h]
    P       = softmax_j(S)                 (T, J)
    c2q     = P @ query                    (T, H)
    b_att   = softmax_t(max_j S)           (T,)
    q2c     = b_att @ context              (H,)
    G       = [context, c2q, context*c2q, context*q2c]   (T, 4H)
    returns (query, G)

Strategy: pure data-parallel over batch, one batch element per NeuronCore
(B = 8 = n_cores, no collectives).  The device runs the O(T*J*H) work:
the cross-similarity matmul, the row softmax, the c2q attention matmul,
and the per-row q2c softmax statistics.  The host does input marshalling
(fp16 cast, context transpose, w_m-scaled query tiles, q_term bias) and
output assembly (the exact f32 context block, the rank-1 q2c reduction,
and the two elementwise products against device-computed factors) --
all O(T*H) or smaller.  Device I/O is fp16; matmuls accumulate in fp32
PSUM.

Math notes used by the kernel:
  * The c_term (ctx@w_c, constant per row t) cancels in softmax_j, so
    P = Etil / sum_j Etil with Etil = exp(cross + q_term[j]).
  * exp(max_j S) = exp(c_term[t]) * max_j Etil[t, :], so the q2c
    softmax weights are M[t] = exp(c_term[t]) * rowmax(Etil) and
    b_att = M / sum(M).
  * |S| <= ~6 so exp() without max-subtraction is exact.

Layout: S is computed transposed (S'^T, [65, t]) so the contraction runs
over H on the partitions; row 64 carries c_term (w_c folded in as a 65th
lhsT column).  exp() is applied with q_term as the per-partition
activation bias (rows 0..63) so row 64 becomes exp(c_term).  A PE
transpose of each E' chunk then yields rowmax/rowsum in [t, 1] layout.
"""

import os
import sys
from contextlib import ExitStack

import numpy as np

for _p in ("/opt/trn_rl_repo", "/root/.axon_site/_ro/trn_rl_repo"):
    if os.path.isdir(_p) and _p not in sys.path:
        sys.path.insert(0, _p)

import concourse.bass as bass  # noqa: E402
import concourse.tile as tile  # noqa: E402
from concourse import bacc, mybir  # noqa: E402
from concourse.bass_utils import run_bass_kernel_spmd  # noqa: E402
from concourse.masks import make_identity  # noqa: E402

B, L, H = 8, 2048, 1024
J = 64          # question_ends + 1
T = L - J       # 1984
NCH = (T + 127) // 128          # 16 t-chunks; last one has 64 rows
NSIG = 4                        # 512-wide superchunks for the cross matmul
N_CORES = 8
KCH = H // 128                  # 8 h-chunks

F32 = mybir.dt.float32
F16 = mybir.dt.float16
ALU = mybir.AluOpType
ACTF = mybir.ActivationFunctionType


def _chunk_rows(c):
    return 64 if c == NCH - 1 else 128


def build_nc():
    nc = bacc.Bacc("TRN2", target_bir_lowering=False, debug=False,
                   num_devices=N_CORES)
    enct = nc.dram_tensor("enct", [128, NSIG, KCH, 512], F16,
                          kind="ExternalInput").ap()
    qh = nc.dram_tensor("qh", [J, H], F16, kind="ExternalInput").ap()
    lhx = nc.dram_tensor("lhx", [128, KCH, 65], F16,
                         kind="ExternalInput").ap()
    qbias = nc.dram_tensor("qbias", [65, 1], F32, kind="ExternalInput").ap()
    g = nc.dram_tensor("g", [128, NCH, H], F16, kind="ExternalOutput").ap()
    mrow = nc.dram_tensor("mrow", [128, NCH], F16, kind="ExternalOutput").ap()
    srow = nc.dram_tensor("srow", [128, NCH], F32, kind="ExternalOutput").ap()
    with tile.TileContext(nc) as tc:
        _emit(tc, enct, qh, lhx, qbias, g, mrow, srow)
    nc.compile()
    return nc


def _emit(tc, enct, qh, lhx, qbias, g, mrow, srow):
    nc = tc.nc
    ctx = ExitStack()
    with ctx:
        consts = ctx.enter_context(tc.tile_pool(name="consts", bufs=1))
        resident = ctx.enter_context(tc.tile_pool(name="resident", bufs=1))
        gsb = ctx.enter_context(tc.tile_pool(name="gsb", bufs=4))
        ctp = ctx.enter_context(tc.tile_pool(name="ctp", bufs=2))
        small = ctx.enter_context(tc.tile_pool(name="small", bufs=6))
        ps_small = ctx.enter_context(
            tc.tile_pool(name="ps_small", bufs=2, space="PSUM"))
        ps_s = ctx.enter_context(
            tc.tile_pool(name="ps_s", bufs=2, space="PSUM"))
        ps_mm = ctx.enter_context(
            tc.tile_pool(name="ps_mm", bufs=4, space="PSUM"))

        # ---- inputs: small tensors first (same HWDGE FIFO) -----------
        query = consts.tile([64, H], F16)
        nc.sync.dma_start(out=query[:, :], in_=qh[:, :])
        lhsT_cross = consts.tile([128, KCH, 65], F16)
        nc.sync.dma_start(out=lhsT_cross[:, :, :], in_=lhx[:, :, :])
        qb = consts.tile([65, 1], F32)
        nc.sync.dma_start(out=qb[:, :], in_=qbias[:, :])



        ident_h = consts.tile([128, 128], F16)
        make_identity(nc, ident_h)

        ep = resident.tile([65, T], F16)          # E' = exp(S' + qbias)
        m_all = resident.tile([128, NCH], F16)    # M[t] per chunk column
        s_all = resident.tile([128, NCH], F32)    # rowsum(Etil) per chunk
        nc.vector.memset(m_all, 0.0)
        nc.vector.memset(s_all, 1.0)

        # HAM warmup: keep the PE busy on junk matmuls while the context
        # DMAs land, so the real cross matmuls start at 2.4 GHz
        warm = consts.tile([128, 512], F16)
        nc.gpsimd.memset(warm, 0.0)
        ones64 = consts.tile([64, 1], F16)
        nc.vector.memset(ones64, 1.0)
        warm_ps = ps_mm.tile([128, 512], F32, tag="mm")
        for _ in range(10):
            nc.tensor.matmul(warm_ps[:, :], ident_h[:, :], warm[:, :],
                             start=True, stop=True)
        wexp = consts.tile([1, 1], F16)
        nc.scalar.activation(out=wexp[:, :], in_=warm[0:1, 0:1],
                             func=ACTF.Exp, bias=0.0, scale=1.0)

        # ---- phase A: software-pipelined superchunks -----------------
        # stage_load(s):  1MB context DMA -> cross matmuls -> exp
        # stage_consume(s): per chunk: E'-transpose stats + c2q + eviction
        # Emitting load(s+2)/exp between consume blocks keeps the PE dense
        # (cross of s+1 fills the bubbles while s's evictions drain).
        def sig_bounds(s):
            c0 = s * 4
            c1 = min(c0 + 4, NCH)
            sw = sum(_chunk_rows(c) for c in range(c0, c1))
            return c0, c1, sw, c0 * 128

        def stage_load(s):
            c0, c1, sw, t0 = sig_bounds(s)
            ct_sig = ctp.tile([128, KCH, 512], F16)
            nc.sync.dma_start(out=ct_sig[:, 0:4, :], in_=enct[:, s, 0:4, :])
            nc.sync.dma_start(out=ct_sig[:, 4:8, :], in_=enct[:, s, 4:8, :])
            s_ps = ps_s.tile([65, 512], F32)
            for k in range(KCH):
                nc.tensor.matmul(s_ps[:, 0:sw], lhsT_cross[:, k, :],
                                 ct_sig[:, k, 0:sw],
                                 start=(k == 0), stop=(k == KCH - 1))
            nc.scalar.activation(out=ep[:, t0:t0 + sw], in_=s_ps[:, 0:sw],
                                 func=ACTF.Exp, bias=qb[:, :], scale=1.0)

        def stage_consume(s):
            c0, c1, sw, t0 = sig_bounds(s)
            g_sig = gsb.tile([128, 4, H], F16)
            for c in range(c0, c1):
                rows = _chunk_rows(c)
                tloc = c * 128
                # transpose E' chunk -> [rows, 65]: per-t stats in [t, 1]
                et_ps = ps_small.tile([128, 512], F16, tag="pss")
                nc.tensor.transpose(et_ps[0:rows, 0:65],
                                    ep[:, tloc:tloc + rows],
                                    ident_h[0:65, 0:65])
                mt = small.tile([128, 1], F16, tag="mt")
                nc.vector.reduce_max(out=mt[0:rows, :],
                                     in_=et_ps[0:rows, 0:64],
                                     axis=mybir.AxisListType.X)
                # M = exp(c_term) * rowmax(Etil)
                nc.vector.tensor_mul(m_all[0:rows, c:c + 1],
                                     mt[0:rows, :], et_ps[0:rows, 64:65])
                nc.vector.reduce_sum(out=s_all[0:rows, c:c + 1],
                                     in_=et_ps[0:rows, 0:64],
                                     axis=mybir.AxisListType.X)

                # c2q (unnormalized; host divides by srow); PSUM eviction
                # split ACT (h0) / DVE (h1)
                for hh in range(2):
                    hs = hh * 512
                    cq_ps = ps_mm.tile([128, 512], F32, tag="mm")
                    nc.tensor.matmul(cq_ps[0:rows, :],
                                     ep[0:64, tloc:tloc + rows],
                                     query[:, hs:hs + 512],
                                     start=True, stop=True)
                    if hh == 0:
                        nc.scalar.activation(
                            out=g_sig[0:rows, c - c0, hs:hs + 512],
                            in_=cq_ps[0:rows, :], func=ACTF.Copy)
                    else:
                        nc.vector.tensor_copy(g_sig[0:rows, c - c0, hs:hs + 512],
                                              cq_ps[0:rows, :])
                if s == NSIG - 1:
                    nc.scalar.dma_start(
                        out=g[0:rows, c:c + 1, :],
                        in_=g_sig[0:rows, c - c0:c - c0 + 1, :])
            if s < NSIG - 1:
                nc.scalar.dma_start(out=g[:, c0:c1, :],
                                    in_=g_sig[:, 0:c1 - c0, :])

        stage_load(0)
        stage_load(1)
        for s in range(NSIG):
            stage_consume(s)
            if s + 2 < NSIG:
                stage_load(s + 2)

        # ---- ship the softmax scale rows -----------------------------
        nc.sync.dma_start(out=mrow[:, :], in_=m_all[:, :])
        nc.sync.dma_start(out=srow[:, :], in_=s_all[:, :])




_NC_CACHE = None


def _get_nc():
    global _NC_CACHE
    if _NC_CACHE is None:
        _NC_CACHE = build_nc()
    return _NC_CACHE


def make_in_maps(encoder_out, w):
    """Host-side input marshalling (fp16 cast, transpose, weight tiles)."""
    w_c, w_q, w_m = w[:H], w[H:2 * H], w[2 * H:]
    enc_h = encoder_out.astype(np.float16)
    in_maps = []
    for b in range(B):
        q32 = encoder_out[b, :J, :]                      # (J, H) f32
        qs_t = (q32 * w_m[None, :]).T.astype(np.float16)  # (H, J)
        lhx = np.empty((128, KCH, 65), np.float16)
        lhx[:, :, 0:64] = qs_t.reshape(KCH, 128, J).transpose(1, 0, 2)
        lhx[:, :, 64] = w_c.reshape(KCH, 128).T.astype(np.float16)
        qbias = np.zeros((65, 1), np.float32)
        qbias[0:64, 0] = q32 @ w_q
        ctx_t = np.zeros((H, NSIG * 512), np.float16)
        ctx_t[:, :T] = enc_h[b, J:, :].T
        enct_r = np.ascontiguousarray(
            ctx_t.reshape(KCH, 128, NSIG, 512).transpose(1, 2, 0, 3))
        in_maps.append({
            "enct": enct_r,
            "qh": np.ascontiguousarray(enc_h[b, :J, :]),
            "lhx": lhx,
            "qbias": qbias,
        })
    return in_maps


def assemble(encoder_out, results):
    """Host-side output assembly from device factors."""
    query = np.ascontiguousarray(encoder_out[:, :J, :])
    G = np.empty((B, T, 4 * H), np.float32)
    for b in range(B):
        ctx_b = encoder_out[b, J:, :]
        s = results[b]["srow"].astype(np.float32).T.ravel()[:T]
        g_r = results[b]["g"].transpose(1, 0, 2).reshape(NCH * 128, H)[:T]
        c2q = g_r.astype(np.float32) / s[:, None]
        m = results[b]["mrow"].astype(np.float32).T.ravel()[:T]
        b_att = m / m.sum()
        q2c = b_att @ ctx_b
        G[b, :, 0:H] = ctx_b
        G[b, :, H:2 * H] = c2q
        G[b, :, 2 * H:3 * H] = ctx_b * c2q
        G[b, :, 3 * H:4 * H] = ctx_b * q2c[None, :]
    return (query, G)


def kernel(encoder_out, w, question_ends):
    encoder_out = np.asarray(encoder_out, dtype=np.float32)
    w = np.asarray(w, dtype=np.float32)
    j = int(question_ends) + 1
    assert j == J and encoder_out.shape == (B, L, H) and w.shape == (3 * H,)

    nc = _get_nc()
    in_maps = make_in_maps(encoder_out, w)
    res = run_bass_kernel_spmd(nc, in_maps, core_ids=list(range(N_CORES)))
    return assemble(encoder_out, res.results)


if __name__ == "__main__":
    x = np.random.randn(B, L, H).astype(np.float32)
    wv = (np.random.randn(3 * H) / np.sqrt(3 * H)).astype(np.float32)
    q, G = kernel(x, wv, np.int64(63))
    print("query", q.shape, "G", G.shape)


# revision 28
# speedup vs baseline: 1.1919x; 1.1919x over previous
"""Trainium2 Bass kernel for nn_BiAttention (dense_transformer).

Reference computation (per batch b, with J = question_ends+1 = 64,
T = L - J = 1984, H = 1024):
    query   = enc[:J]                      (J, H)
    context = enc[J:]                      (T, H)
    w_c, w_q, w_m = w[:H], w[H:2H], w[2H:]
    S[t,j]  = ctx[t]@w_c + q[j]@w_q + sum_h ctx[t,h]*w_m[h]*q[j,h]
    P       = softmax_j(S)                 (T, J)
    c2q     = P @ query                    (T, H)
    b_att   = softmax_t(max_j S)           (T,)
    q2c     = b_att @ context              (H,)
    G       = [context, c2q, context*c2q, context*q2c]   (T, 4H)
    returns (query, G)

Strategy: pure data-parallel over batch, one batch element per NeuronCore
(B = 8 = n_cores, no collectives).  The device runs the O(T*J*H) work:
the cross-similarity matmul, the row softmax, the c2q attention matmul,
and the per-row q2c softmax statistics.  The host does input marshalling
(fp16 cast, context transpose, w_m-scaled query tiles, q_term bias) and
output assembly (the exact f32 context block, the rank-1 q2c reduction,
and the two elementwise products against device-computed factors) --
all O(T*H) or smaller.  Device I/O is fp16; matmuls accumulate in fp32
PSUM.

Math notes used by the kernel:
  * The c_term (ctx@w_c, constant per row t) cancels in softmax_j, so
    P = Etil / sum_j Etil with Etil = exp(cross + q_term[j]).
  * exp(max_j S) = exp(c_term[t]) * max_j Etil[t, :], so the q2c
    softmax weights are M[t] = exp(c_term[t]) * rowmax(Etil) and
    b_att = M / sum(M).
  * |S| <= ~6 so exp() without max-subtraction is exact.

Layout: S is computed transposed (S'^T, [65, t]) so the contraction runs
over H on the partitions; row 64 carries c_term (w_c folded in as a 65th
lhsT column).  exp() is applied with q_term as the per-partition
activation bias (rows 0..63) so row 64 becomes exp(c_term).  A PE
transpose of each E' chunk then yields rowmax/rowsum in [t, 1] layout.
"""

import os
import sys
from contextlib import ExitStack

import numpy as np

for _p in ("/opt/trn_rl_repo", "/root/.axon_site/_ro/trn_rl_repo"):
    if os.path.isdir(_p) and _p not in sys.path:
        sys.path.insert(0, _p)

import concourse.bass as bass  # noqa: E402
import concourse.tile as tile  # noqa: E402
from concourse import bacc, mybir  # noqa: E402
from concourse.bass_utils import run_bass_kernel_spmd  # noqa: E402
from concourse.masks import make_identity  # noqa: E402

B, L, H = 8, 2048, 1024
J = 64          # question_ends + 1
T = L - J       # 1984
NCH = (T + 127) // 128          # 16 t-chunks; last one has 64 rows
NSIG = 4                        # 512-wide superchunks for the cross matmul
N_CORES = 8
KCH = H // 128                  # 8 h-chunks

F32 = mybir.dt.float32
F16 = mybir.dt.float16
ALU = mybir.AluOpType
ACTF = mybir.ActivationFunctionType


def _chunk_rows(c):
    return 64 if c == NCH - 1 else 128


def build_nc():
    nc = bacc.Bacc("TRN2", target_bir_lowering=False, debug=False,
                   num_devices=N_CORES)
    enct = nc.dram_tensor("enct", [128, NSIG, KCH, 512], F16,
                          kind="ExternalInput").ap()
    qh = nc.dram_tensor("qh", [J, H], F16, kind="ExternalInput").ap()
    lhx = nc.dram_tensor("lhx", [128, KCH, 65], F16,
                         kind="ExternalInput").ap()
    qbias = nc.dram_tensor("qbias", [65, 1], F32, kind="ExternalInput").ap()
    g = nc.dram_tensor("g", [128, NCH, H], F16, kind="ExternalOutput").ap()
    mrow = nc.dram_tensor("mrow", [128, NCH], F16, kind="ExternalOutput").ap()
    srow = nc.dram_tensor("srow", [128, NCH], F32, kind="ExternalOutput").ap()
    with tile.TileContext(nc) as tc:
        _emit(tc, enct, qh, lhx, qbias, g, mrow, srow)
    nc.compile()
    return nc


def _emit(tc, enct, qh, lhx, qbias, g, mrow, srow):
    nc = tc.nc
    ctx = ExitStack()
    with ctx:
        consts = ctx.enter_context(tc.tile_pool(name="consts", bufs=1))
        resident = ctx.enter_context(tc.tile_pool(name="resident", bufs=1))
        gsb = ctx.enter_context(tc.tile_pool(name="gsb", bufs=4))
        ctp = ctx.enter_context(tc.tile_pool(name="ctp", bufs=2))
        small = ctx.enter_context(tc.tile_pool(name="small", bufs=6))
        ps_small = ctx.enter_context(
            tc.tile_pool(name="ps_small", bufs=2, space="PSUM"))
        ps_s = ctx.enter_context(
            tc.tile_pool(name="ps_s", bufs=2, space="PSUM"))
        ps_mm = ctx.enter_context(
            tc.tile_pool(name="ps_mm", bufs=4, space="PSUM"))

        # ---- inputs: small tensors first (same HWDGE FIFO) -----------
        query = consts.tile([64, H], F16)
        nc.sync.dma_start(out=query[:, :], in_=qh[:, :])
        lhsT_cross = consts.tile([128, KCH, 65], F16)
        nc.sync.dma_start(out=lhsT_cross[:, :, :], in_=lhx[:, :, :])
        qb = consts.tile([65, 1], F32)
        nc.sync.dma_start(out=qb[:, :], in_=qbias[:, :])



        ident_h = consts.tile([128, 128], F16)
        make_identity(nc, ident_h)

        ep = resident.tile([65, T], F16)          # E' = exp(S' + qbias)
        m_all = resident.tile([128, NCH], F16)    # M[t] per chunk column
        s_all = resident.tile([128, NCH], F32)    # rowsum(Etil) per chunk
        nc.vector.memset(m_all, 0.0)
        nc.vector.memset(s_all, 1.0)

        # HAM warmup: keep the PE busy on junk matmuls while the context
        # DMAs land, so the real cross matmuls start at 2.4 GHz
        warm = consts.tile([128, 512], F16)
        nc.gpsimd.memset(warm, 0.0)
        ones64 = consts.tile([64, 1], F16)
        nc.vector.memset(ones64, 1.0)
        warm_ps = ps_mm.tile([128, 512], F32, tag="mm")
        for _ in range(12):
            nc.tensor.matmul(warm_ps[:, :], ident_h[:, :], warm[:, :],
                             start=True, stop=True)
        wexp = consts.tile([1, 1], F16)
        nc.scalar.activation(out=wexp[:, :], in_=warm[0:1, 0:1],
                             func=ACTF.Exp, bias=0.0, scale=1.0)

        # ---- phase A: software-pipelined superchunks -----------------
        # stage_load(s):  1MB context DMA -> cross matmuls -> exp
        # stage_consume(s): per chunk: E'-transpose stats + c2q + eviction
        # Emitting load(s+2)/exp between consume blocks keeps the PE dense
        # (cross of s+1 fills the bubbles while s's evictions drain).
        def sig_bounds(s):
            c0 = s * 4
            c1 = min(c0 + 4, NCH)
            sw = sum(_chunk_rows(c) for c in range(c0, c1))
            return c0, c1, sw, c0 * 128

        def stage_load(s):
            c0, c1, sw, t0 = sig_bounds(s)
            ct_sig = ctp.tile([128, KCH, 512], F16)
            nc.sync.dma_start(out=ct_sig[:, 0:4, :], in_=enct[:, s, 0:4, :])
            nc.sync.dma_start(out=ct_sig[:, 4:8, :], in_=enct[:, s, 4:8, :])
            s_ps = ps_s.tile([65, 512], F32)
            for k in range(KCH):
                nc.tensor.matmul(s_ps[:, 0:sw], lhsT_cross[:, k, :],
                                 ct_sig[:, k, 0:sw],
                                 start=(k == 0), stop=(k == KCH - 1))
            nc.scalar.activation(out=ep[:, t0:t0 + sw], in_=s_ps[:, 0:sw],
                                 func=ACTF.Exp, bias=qb[:, :], scale=1.0)

        def stage_consume(s):
            c0, c1, sw, t0 = sig_bounds(s)
            g_sig = gsb.tile([128, 4, H], F16)
            for c in range(c0, c1):
                rows = _chunk_rows(c)
                tloc = c * 128
                # transpose E' chunk -> [rows, 65]: per-t stats in [t, 1]
                et_ps = ps_small.tile([128, 512], F16, tag="pss")
                nc.tensor.transpose(et_ps[0:rows, 0:65],
                                    ep[:, tloc:tloc + rows],
                                    ident_h[0:65, 0:65])
                mt = small.tile([128, 1], F16, tag="mt")
                nc.vector.reduce_max(out=mt[0:rows, :],
                                     in_=et_ps[0:rows, 0:64],
                                     axis=mybir.AxisListType.X)
                # M = exp(c_term) * rowmax(Etil)
                nc.vector.tensor_mul(m_all[0:rows, c:c + 1],
                                     mt[0:rows, :], et_ps[0:rows, 64:65])
                nc.vector.reduce_sum(out=s_all[0:rows, c:c + 1],
                                     in_=et_ps[0:rows, 0:64],
                                     axis=mybir.AxisListType.X)

                # c2q (unnormalized; host divides by srow); PSUM eviction
                # split ACT (h0) / DVE (h1)
                for hh in range(2):
                    hs = hh * 512
                    cq_ps = ps_mm.tile([128, 512], F32, tag="mm")
                    nc.tensor.matmul(cq_ps[0:rows, :],
                                     ep[0:64, tloc:tloc + rows],
                                     query[:, hs:hs + 512],
                                     start=True, stop=True)
                    if hh == 0:
                        nc.scalar.activation(
                            out=g_sig[0:rows, c - c0, hs:hs + 512],
                            in_=cq_ps[0:rows, :], func=ACTF.Copy)
                    else:
                        nc.vector.tensor_copy(g_sig[0:rows, c - c0, hs:hs + 512],
                                              cq_ps[0:rows, :])
                if s == NSIG - 1:
                    nc.scalar.dma_start(
                        out=g[0:rows, c:c + 1, :],
                        in_=g_sig[0:rows, c - c0:c - c0 + 1, :])
            if s < NSIG - 1:
                nc.scalar.dma_start(out=g[:, c0:c1, :],
                                    in_=g_sig[:, 0:c1 - c0, :])

        stage_load(0)
        stage_load(1)
        for s in range(NSIG):
            stage_consume(s)
            if s + 2 < NSIG:
                stage_load(s + 2)

        # ---- ship the softmax scale rows -----------------------------
        nc.sync.dma_start(out=mrow[:, :], in_=m_all[:, :])
        nc.sync.dma_start(out=srow[:, :], in_=s_all[:, :])




_NC_CACHE = None


def _get_nc():
    global _NC_CACHE
    if _NC_CACHE is None:
        _NC_CACHE = build_nc()
    return _NC_CACHE


def make_in_maps(encoder_out, w):
    """Host-side input marshalling (fp16 cast, transpose, weight tiles)."""
    w_c, w_q, w_m = w[:H], w[H:2 * H], w[2 * H:]
    enc_h = encoder_out.astype(np.float16)
    in_maps = []
    for b in range(B):
        q32 = encoder_out[b, :J, :]                      # (J, H) f32
        qs_t = (q32 * w_m[None, :]).T.astype(np.float16)  # (H, J)
        lhx = np.empty((128, KCH, 65), np.float16)
        lhx[:, :, 0:64] = qs_t.reshape(KCH, 128, J).transpose(1, 0, 2)
        lhx[:, :, 64] = w_c.reshape(KCH, 128).T.astype(np.float16)
        qbias = np.zeros((65, 1), np.float32)
        qbias[0:64, 0] = q32 @ w_q
        ctx_t = np.zeros((H, NSIG * 512), np.float16)
        ctx_t[:, :T] = enc_h[b, J:, :].T
        enct_r = np.ascontiguousarray(
            ctx_t.reshape(KCH, 128, NSIG, 512).transpose(1, 2, 0, 3))
        in_maps.append({
            "enct": enct_r,
            "qh": np.ascontiguousarray(enc_h[b, :J, :]),
            "lhx": lhx,
            "qbias": qbias,
        })
    return in_maps


def assemble(encoder_out, results):
    """Host-side output assembly from device factors."""
    query = np.ascontiguousarray(encoder_out[:, :J, :])
    G = np.empty((B, T, 4 * H), np.float32)
    for b in range(B):
        ctx_b = encoder_out[b, J:, :]
        s = results[b]["srow"].astype(np.float32).T.ravel()[:T]
        g_r = results[b]["g"].transpose(1, 0, 2).reshape(NCH * 128, H)[:T]
        c2q = g_r.astype(np.float32) / s[:, None]
        m = results[b]["mrow"].astype(np.float32).T.ravel()[:T]
        b_att = m / m.sum()
        q2c = b_att @ ctx_b
        G[b, :, 0:H] = ctx_b
        G[b, :, H:2 * H] = c2q
        G[b, :, 2 * H:3 * H] = ctx_b * c2q
        G[b, :, 3 * H:4 * H] = ctx_b * q2c[None, :]
    return (query, G)


def kernel(encoder_out, w, question_ends):
    encoder_out = np.asarray(encoder_out, dtype=np.float32)
    w = np.asarray(w, dtype=np.float32)
    j = int(question_ends) + 1
    assert j == J and encoder_out.shape == (B, L, H) and w.shape == (3 * H,)

    nc = _get_nc()
    in_maps = make_in_maps(encoder_out, w)
    res = run_bass_kernel_spmd(nc, in_maps, core_ids=list(range(N_CORES)))
    return assemble(encoder_out, res.results)


if __name__ == "__main__":
    x = np.random.randn(B, L, H).astype(np.float32)
    wv = (np.random.randn(3 * H) / np.sqrt(3 * H)).astype(np.float32)
    q, G = kernel(x, wv, np.int64(63))
    print("query", q.shape, "G", G.shape)
